# revision 1
# baseline (speedup 1.0000x reference)
"""GCN edge-aggregation kernel for 8 Trainium2 NeuronCores.

Math (see nn_GCNEdge): h = relu((segment_sum(edge_data, dst) / max(count,1)) @ W.T + b)

Strategy
--------
Host-side (sharding/layout only — all arithmetic happens on device):
  * Nodes are split contiguously across the 8 cores (12544 = 98 blocks of 128
    nodes per core; 8*12544 = 100352 >= 100000).
  * Each edge is routed to the core/block owning its destination node (CSR-style
    destination binning).  Within a block, edges occupy sequential slots; each
    block is padded to K_CHUNKS*128 slots so the device program is data-independent.
  * Edge features are shipped as a bf16 hi/lo pair (hi = bf16(x),
    lo = bf16(x - hi)) so the on-device f32-accumulated matmuls reconstruct
    ~fp32 precision while streaming at bf16 rates.  A constant-1 column rides
    along for the degree counts.

Device-side (per core, per 128-node block):
  * One-hot matrix of local node ids (DVE is_equal against an iota row),
  * PE matmul-accumulate onehot.T @ [x_hi | 1 | x_lo | 0] into PSUM -> per-node
    feature sums (hi+lo parts) and counts,
  * mean = sums * reciprocal(max(count, 1)),
  * PE transpose, then out = relu(W @ agg.T + b) via a second matmul with the
    (pre-transposed) weight as the stationary operand; output stays transposed
    [out_feat, node] and is un-transposed on the host.

No collectives are needed: output shards are disjoint.
"""

import numpy as np
import ml_dtypes

BF16 = ml_dtypes.bfloat16

N_NODES = 100000
N_EDGES = 1600000
F = 128
N_CORES = 8
BLK = 128                       # nodes per block
BLOCKS_PER_CORE = 98
TOTAL_BLOCKS = N_CORES * BLOCKS_PER_CORE        # 784
NODES_PER_CORE = BLOCKS_PER_CORE * BLK          # 12544
K_CHUNKS = 18                   # 128-edge chunks per block (capacity 2304 edges)

_module_cache = {}


def _build_module(K):
    import concourse.mybir as mybir
    import concourse.tile as tile
    from concourse import bacc

    f32 = mybir.dt.float32
    bf16 = mybir.dt.bfloat16
    RB = K * 128                 # edge slots per block
    SLOTS = BLOCKS_PER_CORE * RB

    nc = bacc.Bacc("TRN2", target_bir_lowering=False, debug=False)
    # xe rows are (block, partition); each row is that partition's K chunks of
    # 258 bf16 values laid contiguously -> 9KB-contiguous DMA descriptors.
    xe = nc.dram_tensor("xe", [BLOCKS_PER_CORE * 128, K * 258], bf16, kind="ExternalInput")
    lid = nc.dram_tensor("lid", [128, BLOCKS_PER_CORE * K], bf16, kind="ExternalInput")
    wt = nc.dram_tensor("wt", [128, 128], f32, kind="ExternalInput")
    bias = nc.dram_tensor("bias", [128, 1], f32, kind="ExternalInput")
    ident = nc.dram_tensor("ident", [128, 128], f32, kind="ExternalInput")
    # iota value pattern tiled K times: iotar[p, c*128 + f] = f
    iotar = nc.dram_tensor("iotar", [128, K * 128], bf16, kind="ExternalInput")
    out = nc.dram_tensor("out", [128, BLOCKS_PER_CORE * 128], f32, kind="ExternalOutput")

    xe_ap = xe.ap()
    out_ap = out.ap()

    with tile.TileContext(nc) as tc:
        with (
            tc.tile_pool(name="const", bufs=1) as cpool,
            tc.tile_pool(name="xp", bufs=6) as xpool,
            tc.tile_pool(name="ohp", bufs=8) as ohpool,
            tc.tile_pool(name="ep", bufs=3) as epool,
            tc.tile_pool(name="psS", bufs=4, space="PSUM") as psS,
            tc.tile_pool(name="psT", bufs=2, space="PSUM") as psT,
            tc.tile_pool(name="psO", bufs=2, space="PSUM") as psO,
        ):
            wt_t = cpool.tile([128, 128], f32)
            nc.sync.dma_start(wt_t[:], wt.ap()[:])
            bias_t = cpool.tile([128, 1], f32)
            nc.sync.dma_start(bias_t[:], bias.ap()[:])
            id_t = cpool.tile([128, 128], f32)
            nc.sync.dma_start(id_t[:], ident.ap()[:])
            iotar_t = cpool.tile([128, K * 128], bf16)
            nc.sync.dma_start(iotar_t[:], iotar.ap()[:])
            lid_t = cpool.tile([128, BLOCKS_PER_CORE * K], bf16)
            nc.sync.dma_start(lid_t[:], lid.ap()[:])

            group_pT = {}

            def emit_matmuls(b, xt, oh):
                ps = psS.tile([128, 258], f32, name=f"ps{b}", tag="ps")
                for c in range(K):
                    nc.tensor.matmul(
                        ps[:],
                        lhsT=oh[:, c * 128:(c + 1) * 128],
                        rhs=xt[:, c * 258:(c + 1) * 258],
                        start=(c == 0),
                        stop=(c == K - 1),
                    )
                return ps

            def emit_pscopy(b, ps):
                # Drain PSUM to SBUF with a single ACT copy (emitted one block
                # after the accumulation finished, so the ACT queue never
                # blocks on it) — frees the PSUM bank early; the lagged
                # epilogue then reads SBUF only.
                s_sb = epool.tile([128, 257], f32, name=f"s{b}", tag="s_sb", bufs=5)
                nc.scalar.copy(s_sb[:], ps[:, 0:257])
                return s_sb

            def emit_epilogue(b, ps):
                # counts live in ps[:,128] (the lo-side count column is all
                # zeros by construction), so no hi+lo add is needed for them.
                # No max(count,1) guard: the host guarantees every real node
                # has count > 0 (injecting 1e-30-weight phantom edges if
                # needed); padding nodes divide by zero -> NaN columns that
                # the host slices off.  Keeping DVE's per-block work to this
                # single tiny op is what lets the wide one-hot builds stream.
                rec = epool.tile([128, 1], f32, name=f"rec{b}", tag="rec")
                nc.vector.reciprocal(rec[:], ps[:, 128:129])
                # agg = (S_hi + S_lo)/count: t1 = S_hi*rec on ACT, then one
                # fused DVE op: agg = S_lo*rec + t1.
                t1 = epool.tile([128, 128], f32, name=f"t1{b}", tag="t1")
                nc.scalar.activation(
                    t1[:], ps[:, 0:128],
                    mybir.ActivationFunctionType.Copy, scale=rec[:, 0:1],
                )
                agg = epool.tile([128, 128], f32, name=f"agg{b}", tag="agg")
                nc.vector.scalar_tensor_tensor(
                    out=agg[:],
                    in0=ps[:, 129:257],
                    scalar=rec[:, 0:1],
                    in1=t1[:],
                    op0=mybir.AluOpType.mult,
                    op1=mybir.AluOpType.add,
                )
                # NOTE: `ps` here is the SBUF-staged copy (s_sb), not PSUM.
                j = b % 4
                if j == 0:
                    group_pT["t"] = psT.tile([128, 512], f32, name=f"pT{b}", tag="pT")
                pT = group_pT["t"]
                nc.tensor.transpose(pT[:, j * 128:(j + 1) * 128], agg[:], id_t[:])
                if j == 3 or b == BLOCKS_PER_CORE - 1:
                    g0 = (b // 4) * 4
                    gw = (b + 1 - g0) * 128
                    aggT = epool.tile([128, 512], f32, name=f"aggT{b}", tag="aggT", bufs=2)
                    nc.scalar.copy(aggT[:, 0:gw], pT[:, 0:gw])
                    pO = psO.tile([128, 512], f32, name=f"pO{b}", tag="pO")
                    nc.tensor.matmul(
                        pO[:, 0:gw], lhsT=wt_t[:], rhs=aggT[:, 0:gw],
                        start=True, stop=True,
                    )
                    ot = epool.tile([128, 512], f32, name=f"ot{b}", tag="ot", bufs=2)
                    nc.scalar.activation(
                        ot[:, 0:gw], pO[:, 0:gw],
                        mybir.ActivationFunctionType.Relu,
                        bias=bias_t[:, 0:1], scale=1.0,
                    )
                    nc.sync.dma_start(out_ap[:, g0 * 128:(b + 1) * 128], ot[:, 0:gw])

            # Software-pipelined emission. Every engine queue is strict
            # in-order, so an op gated on *fresh* upstream state stalls the
            # whole queue behind it. Stagger each stage so, by the time a
            # queue reaches an op, its dependencies are blocks old:
            #   iter b:  DMA xt(b) | one-hot TT(b) | PE matmuls(b-1)
            #            | PSUM->SBUF drain of (b-2) | epilogue of (b-5)
            pending = {}
            pending_ps = {}
            pending_s = {}
            for b in range(BLOCKS_PER_CORE):
                xt = xpool.tile([128, K * 258], bf16, name=f"xt{b}", tag="xt")
                nc.sync.dma_start(xt[:], xe_ap[b * 128:(b + 1) * 128, :])
                oh = ohpool.tile([128, K * 128], bf16, name=f"oh{b}", tag="oh")
                nc.vector.tensor_tensor(
                    out=oh[:].rearrange("p (c f) -> p c f", c=K),
                    in0=iotar_t[:].rearrange("p (c f) -> p c f", c=K),
                    in1=lid_t[:, b * K:(b + 1) * K].to_broadcast([128, K, 128]),
                    op=mybir.AluOpType.is_equal,
                )
                pending[b] = (xt, oh)
                if b >= 1:
                    pending_ps[b - 1] = emit_matmuls(b - 1, *pending.pop(b - 1))
                if b >= 2:
                    pending_s[b - 2] = emit_pscopy(b - 2, pending_ps.pop(b - 2))
                if b >= 5:
                    emit_epilogue(b - 5, pending_s.pop(b - 5))
            last = BLOCKS_PER_CORE - 1
            pending_ps[last] = emit_matmuls(last, *pending.pop(last))
            for bb in sorted(pending_ps):
                pending_s[bb] = emit_pscopy(bb, pending_ps.pop(bb))
            for bb in sorted(pending_s):
                emit_epilogue(bb, pending_s.pop(bb))

    nc.compile()
    return nc


def _get_module(K):
    if K not in _module_cache:
        _module_cache[K] = _build_module(K)
    return _module_cache[K]


def prepare_inputs(edge_data, dst, W, b):
    """Host-side sharding: route each edge to the core/block owning dst."""
    edge_data = np.asarray(edge_data, dtype=np.float32)
    dst = np.asarray(dst)
    W = np.asarray(W, dtype=np.float32)
    b = np.asarray(b, dtype=np.float32)
    E = dst.shape[0]

    # The device kernel divides by the raw count (no max(count,1) guard).
    # Give any zero-degree real node a phantom edge with zero features and a
    # 1e-30 "count" weight: sums stay exactly 0, so mean = 0/1e-30 = 0, which
    # matches the reference's 0/max(0,1).
    node_cnt = np.bincount(dst, minlength=N_NODES)[:N_NODES]
    zeros = np.nonzero(node_cnt == 0)[0]
    n_real = E
    if len(zeros):
        dst = np.concatenate([dst, zeros.astype(dst.dtype)])
        E = dst.shape[0]

    blk = (dst.astype(np.int64)) >> 7                 # destination block id
    cnt = np.bincount(blk, minlength=TOTAL_BLOCKS)
    K = max(K_CHUNKS, int(np.ceil(cnt.max() / 128)))
    RB = K * 128
    TOT = TOTAL_BLOCKS * RB

    starts = np.zeros(TOTAL_BLOCKS, np.int64)
    np.cumsum(cnt[:-1], out=starts[1:])
    order = np.argsort(blk, kind="stable")
    rank = np.empty(E, np.int64)
    rank[order] = np.arange(E, dtype=np.int64) - np.repeat(starts, cnt)
    slot = blk * RB + rank

    X = np.zeros((TOT, 258), BF16)
    xh = edge_data.astype(BF16)
    X[slot[:n_real], 0:128] = xh
    X[slot[:n_real], 128] = BF16(1.0)
    X[slot[:n_real], 129:257] = (edge_data - xh.astype(np.float32)).astype(BF16)
    if len(zeros):
        X[slot[n_real:], 128] = BF16(1e-30)
    # [block, chunk, partition, feat] -> [block, partition, chunk*feat] so each
    # SBUF partition's data is one long contiguous HBM run (big DMA descriptors).
    X = np.ascontiguousarray(
        X.reshape(TOTAL_BLOCKS, K, 128, 258).transpose(0, 2, 1, 3)
    ).reshape(N_CORES, BLOCKS_PER_CORE * 128, K * 258)

    lid_f = np.full(TOT, -1.0, np.float32)
    lid_f[slot] = (dst & 127).astype(np.float32)
    lid_all = (
        lid_f.reshape(N_CORES, BLOCKS_PER_CORE, K, 128)
        .transpose(0, 3, 1, 2)
        .reshape(N_CORES, 128, BLOCKS_PER_CORE * K)
        .astype(BF16)
    )
    wt = np.ascontiguousarray(W.T)
    bias = np.ascontiguousarray(b.reshape(128, 1))
    ident = np.eye(128, dtype=np.float32)
    iotar = np.ascontiguousarray(
        np.broadcast_to(
            np.arange(128, dtype=np.float32), (128, K, 128)
        ).reshape(128, K * 128)
    ).astype(BF16)

    in_maps = [
        {
            "xe": np.ascontiguousarray(X[c]),
            "lid": np.ascontiguousarray(lid_all[c]),
            "wt": wt,
            "bias": bias,
            "ident": ident,
            "iotar": iotar,
        }
        for c in range(N_CORES)
    ]
    return K, in_maps


def run(edge_data, dst, W, b, trace=False, tmpdir=None):
    from concourse.bass_utils import run_bass_kernel_spmd

    K, in_maps = prepare_inputs(edge_data, dst, W, b)
    nc = _get_module(K)
    res = run_bass_kernel_spmd(
        nc, in_maps, core_ids=list(range(N_CORES)), trace=trace, tmpdir=tmpdir,
    )
    outs = [res.results[c]["out"].T for c in range(N_CORES)]   # [12544, 128] each
    full = np.concatenate(outs, axis=0)[:N_NODES]
    return np.ascontiguousarray(full, dtype=np.float32), res


def kernel(edge_data, dst, W, b):
    out, _ = run(edge_data, dst, W, b, trace=False)
    return out



# revision 2
# speedup vs baseline: 1.9453x; 1.9453x over previous
"""GCN edge-aggregation kernel for 8 Trainium2 NeuronCores.

Math (see nn_GCNEdge): h = relu((segment_sum(edge_data, dst) / max(count,1)) @ W.T + b)

Strategy
--------
Host-side (sharding/layout only — all arithmetic happens on device):
  * Nodes are dealt into 784 blocks of <=128 nodes with a serpentine deal over
    degree-sorted nodes (plus a small swap-repair pass), equalizing per-block
    edge counts so every block fits K_CHUNKS*128 = 2048 edge slots.  8 cores
    x 98 blocks; the output is un-permuted on the host at the end.
  * Each edge is routed to the core/block owning its destination node; within
    a block, edges occupy sequential slots padded to K*128.
  * Edge features ship as single bf16 (the rel-err budget is 2e-2; bf16 with
    f32 PSUM accumulation lands ~1e-3) with a constant-1 column riding along
    for the degree counts.  Blocks are paired so each input DMA moves ~1 MiB.

Device-side (per core, per 128-node block):
  * One-hot of local node ids via one DVE is_equal in [p, n*K+c] layout: the
    lid operand broadcasts along n with a stride-1 innermost c axis, so all
    operands are packed 2-byte APs and the op runs in 2x DVE mode (the naive
    [p, c*128+n] layout has a stride-0 innermost broadcast -> 1x mode).
  * PE matmul-accumulate onehot.T @ [x | 1] into PSUM (chunk c of the one-hot
    is the strided slice [:, c, :]) -> per-node sums + counts,
  * mean = sums * reciprocal(count) (phantom 1e-30-count edges keep real
    zero-degree nodes finite; padding nodes produce NaNs the host drops),
  * PE transpose, then out = relu(W @ agg.T + b) in bf16 (fp32 PE matmuls run
    at 1/4 rate), output stays [out_feat, node] bf16 and is un-transposed,
    un-permuted, and cast to f32 on the host.

No collectives: output shards are disjoint.
"""

import numpy as np
import ml_dtypes

BF16 = ml_dtypes.bfloat16

N_NODES = 100000
N_EDGES = 1600000
F = 128
N_CORES = 8
BLK = 128                       # nodes per block
BLOCKS_PER_CORE = 98
TOTAL_BLOCKS = N_CORES * BLOCKS_PER_CORE        # 784
NODES_PER_CORE = BLOCKS_PER_CORE * BLK          # 12544
K_CHUNKS = 16                   # 128-edge chunks per block (capacity 2048)

_module_cache = {}


def _build_module(K):
    import concourse.mybir as mybir
    import concourse.tile as tile
    from concourse import bacc

    f32 = mybir.dt.float32
    bf16 = mybir.dt.bfloat16
    B = BLOCKS_PER_CORE
    NPAIR = B // 2
    W129 = K * 129               # per-block row width (128 feats + count col)

    nc = bacc.Bacc("TRN2", target_bir_lowering=False, debug=False)
    # xe rows are (pair, partition); each row holds both blocks' K chunks of
    # 129 bf16 values contiguously -> ~1 MiB DMAs with 8.25KB-contiguous lines.
    xe = nc.dram_tensor("xe", [NPAIR * 128, 2 * W129], bf16, kind="ExternalInput")
    lid = nc.dram_tensor("lid", [128, B * K], bf16, kind="ExternalInput")
    # iotar[p, n*K + c] = n
    iotar = nc.dram_tensor("iotar", [128, 128 * K], bf16, kind="ExternalInput")
    wt = nc.dram_tensor("wt", [128, 128], bf16, kind="ExternalInput")
    bias = nc.dram_tensor("bias", [128, 1], f32, kind="ExternalInput")
    ident = nc.dram_tensor("ident", [128, 128], f32, kind="ExternalInput")
    out = nc.dram_tensor("out", [128, B * 128], bf16, kind="ExternalOutput")

    xe_ap = xe.ap()
    out_ap = out.ap()

    with tile.TileContext(nc) as tc:
        with (
            tc.tile_pool(name="const", bufs=1) as cpool,
            tc.tile_pool(name="xp", bufs=4) as xpool,
            tc.tile_pool(name="ohp", bufs=6) as ohpool,
            tc.tile_pool(name="ep", bufs=3) as epool,
            tc.tile_pool(name="psS", bufs=4, space="PSUM") as psS,
            tc.tile_pool(name="psT", bufs=2, space="PSUM") as psT,
            tc.tile_pool(name="psO", bufs=2, space="PSUM") as psO,
        ):
            wt_t = cpool.tile([128, 128], bf16)
            nc.sync.dma_start(wt_t[:], wt.ap()[:])
            bias_t = cpool.tile([128, 1], f32)
            nc.sync.dma_start(bias_t[:], bias.ap()[:])
            id_t = cpool.tile([128, 128], f32)
            nc.sync.dma_start(id_t[:], ident.ap()[:])
            iotar_t = cpool.tile([128, 128 * K], bf16)
            nc.sync.dma_start(iotar_t[:], iotar.ap()[:])
            lid_t = cpool.tile([128, B * K], bf16)
            nc.sync.dma_start(lid_t[:], lid.ap()[:])

            group_pT = {}

            def emit_oh(b):
                oh = ohpool.tile([128, 128 * K], bf16, name=f"oh{b}", tag="oh")
                nc.vector.tensor_tensor(
                    out=oh[:].rearrange("p (n c) -> p n c", c=K),
                    in0=iotar_t[:].rearrange("p (n c) -> p n c", c=K),
                    in1=lid_t[:, b * K:(b + 1) * K].unsqueeze(1).to_broadcast(
                        [128, 128, K]
                    ),
                    op=mybir.AluOpType.is_equal,
                )
                return oh

            def emit_matmuls(b, xt, oh):
                ps = psS.tile([128, 129], f32, name=f"ps{b}", tag="ps")
                half = (b % 2) * W129
                ohc = oh[:].rearrange("p (n c) -> p c n", c=K)
                for c in range(K):
                    nc.tensor.matmul(
                        ps[:],
                        lhsT=ohc[:, c, :],
                        rhs=xt[:, half + c * 129:half + (c + 1) * 129],
                        start=(c == 0),
                        stop=(c == K - 1),
                    )
                return ps

            def emit_drain(b, ps):
                # Drain PSUM to SBUF one block after accumulation finished, so
                # the ACT queue never blocks on it; frees the PSUM bank early.
                s_sb = epool.tile([128, 129], f32, name=f"s{b}", tag="s_sb", bufs=5)
                nc.scalar.copy(s_sb[:], ps[:])
                return s_sb

            def emit_epilogue(b, s):
                # counts live in s[:,128].  No max(count,1) guard: the host
                # guarantees every real node has count > 0 (1e-30-weight
                # phantom edges); padding nodes divide by zero -> NaN columns
                # the host drops.
                rec = epool.tile([128, 1], f32, name=f"rec{b}", tag="rec")
                nc.vector.reciprocal(rec[:], s[:, 128:129])
                agg = epool.tile([128, 128], f32, name=f"agg{b}", tag="agg")
                nc.scalar.activation(
                    agg[:], s[:, 0:128],
                    mybir.ActivationFunctionType.Copy, scale=rec[:, 0:1],
                )
                j = b % 4
                if j == 0:
                    group_pT["t"] = psT.tile([128, 512], f32, name=f"pT{b}", tag="pT")
                pT = group_pT["t"]
                nc.tensor.transpose(pT[:, j * 128:(j + 1) * 128], agg[:], id_t[:])
                if j == 3 or b == B - 1:
                    g0 = (b // 4) * 4
                    gw = (b + 1 - g0) * 128
                    aggT = epool.tile([128, 512], bf16, name=f"aggT{b}", tag="aggT", bufs=2)
                    nc.scalar.copy(aggT[:, 0:gw], pT[:, 0:gw])
                    pO = psO.tile([128, 512], f32, name=f"pO{b}", tag="pO")
                    nc.tensor.matmul(
                        pO[:, 0:gw], lhsT=wt_t[:], rhs=aggT[:, 0:gw],
                        start=True, stop=True,
                    )
                    ot = epool.tile([128, 512], bf16, name=f"ot{b}", tag="ot", bufs=2)
                    nc.scalar.activation(
                        ot[:, 0:gw], pO[:, 0:gw],
                        mybir.ActivationFunctionType.Relu,
                        bias=bias_t[:, 0:1], scale=1.0,
                    )
                    nc.sync.dma_start(out_ap[:, g0 * 128:(b + 1) * 128], ot[:, 0:gw])

            # Software-pipelined emission; per-engine queues are strict
            # in-order, so each stage's dependencies are blocks old by the
            # time its queue reaches the op:
            #   iter b:  DMA pair(b/2) | one-hot(b) | PE matmuls(b-1)
            #            | PSUM->SBUF drain(b-2) | epilogue(b-5)
            pending = {}
            pending_ps = {}
            pending_s = {}
            xt_of = {}
            for b in range(B):
                if b % 2 == 0:
                    q = b // 2
                    xt = xpool.tile([128, 2 * W129], bf16, name=f"xt{q}", tag="xt")
                    nc.sync.dma_start(xt[:], xe_ap[q * 128:(q + 1) * 128, :])
                    xt_of[b] = xt
                    xt_of[b + 1] = xt
                oh = emit_oh(b)
                pending[b] = (xt_of.pop(b), oh)
                if b >= 1:
                    pending_ps[b - 1] = emit_matmuls(b - 1, *pending.pop(b - 1))
                if b >= 2:
                    pending_s[b - 2] = emit_drain(b - 2, pending_ps.pop(b - 2))
                if b >= 5:
                    emit_epilogue(b - 5, pending_s.pop(b - 5))
            last = B - 1
            pending_ps[last] = emit_matmuls(last, *pending.pop(last))
            for bb in sorted(pending_ps):
                pending_s[bb] = emit_drain(bb, pending_ps.pop(bb))
            for bb in sorted(pending_s):
                emit_epilogue(bb, pending_s.pop(bb))

    nc.compile()
    return nc


def _get_module(K):
    if K not in _module_cache:
        _module_cache[K] = _build_module(K)
    return _module_cache[K]


def _balance_nodes(deg):
    """Assign nodes to TOTAL_BLOCKS bins of <=128 nodes, equalizing per-bin
    edge load.  Returns (node_bin, node_loc, K)."""
    w = np.where(deg == 0, 1, deg).astype(np.int64)   # phantom edge for deg-0
    order = np.argsort(-w, kind="stable")
    S = TOTAL_BLOCKS * BLK
    idx = np.arange(S)
    r = idx // TOTAL_BLOCKS
    k = idx % TOTAL_BLOCKS
    bins = np.where(r % 2 == 0, k, TOTAL_BLOCKS - 1 - k)   # serpentine deal
    node_bin = np.empty(N_NODES, np.int64)
    node_loc = np.empty(N_NODES, np.int64)
    node_bin[order] = bins[:N_NODES]
    node_loc[order] = r[:N_NODES]
    loads = np.zeros(TOTAL_BLOCKS, np.int64)
    np.add.at(loads, node_bin, w)

    cap = K_CHUNKS * BLK
    for _ in range(2000):
        hot = int(loads.argmax())
        if loads[hot] <= cap:
            break
        cold = int(loads.argmin())
        need = int(loads[hot] - cap)
        headroom = int(cap - loads[cold])
        hot_nodes = np.nonzero(node_bin == hot)[0]
        cold_nodes = np.nonzero(node_bin == cold)[0]
        diffs = w[hot_nodes][:, None] - w[cold_nodes][None, :]
        mask = (diffs >= 1) & (diffs <= headroom)
        if not mask.any():
            break
        cand = np.where(mask, np.abs(diffs - need), 1 << 30)
        ai, bi = np.unravel_index(np.argmin(cand), cand.shape)
        na, nb = hot_nodes[ai], cold_nodes[bi]
        node_bin[na], node_bin[nb] = cold, hot
        node_loc[na], node_loc[nb] = node_loc[nb], node_loc[na]
        d = int(w[na] - w[nb])
        loads[hot] -= d
        loads[cold] += d

    K = max(K_CHUNKS, int(np.ceil(loads.max() / BLK)))
    return node_bin, node_loc, K


def prepare_inputs(edge_data, dst, W, b):
    """Host-side sharding: route each edge to the core/block owning dst."""
    edge_data = np.asarray(edge_data, dtype=np.float32)
    dst = np.asarray(dst).astype(np.int64)
    W = np.asarray(W, dtype=np.float32)
    b = np.asarray(b, dtype=np.float32)
    E = dst.shape[0]

    deg = np.bincount(dst, minlength=N_NODES)[:N_NODES]
    node_bin, node_loc, K = _balance_nodes(deg)
    RB = K * 128
    TOT = TOTAL_BLOCKS * RB

    zeros = np.nonzero(deg == 0)[0]
    dst_ext = np.concatenate([dst, zeros]) if len(zeros) else dst
    n_real = E
    Eext = dst_ext.shape[0]

    eblk = node_bin[dst_ext]
    cnt = np.bincount(eblk, minlength=TOTAL_BLOCKS)
    starts = np.zeros(TOTAL_BLOCKS, np.int64)
    np.cumsum(cnt[:-1], out=starts[1:])
    eorder = np.argsort(eblk, kind="stable")
    rank = np.empty(Eext, np.int64)
    rank[eorder] = np.arange(Eext, dtype=np.int64) - np.repeat(starts, cnt)
    slot = eblk * RB + rank

    X = np.zeros((TOT, 129), BF16)
    X[slot[:n_real], 0:128] = edge_data.astype(BF16)
    X[slot[:n_real], 128] = BF16(1.0)
    if len(zeros):
        X[slot[n_real:], 128] = BF16(1e-30)
    # [block, chunk, part, 129] -> [pair, part, 2 * chunk * 129]: each SBUF
    # partition's data is one 8.25KB contiguous HBM run, ~1MiB per DMA.
    Xb = X.reshape(TOTAL_BLOCKS, K, 128, 129).transpose(0, 2, 1, 3)
    Xb = Xb.reshape(N_CORES, BLOCKS_PER_CORE // 2, 2, 128, K * 129)
    xe_all = np.ascontiguousarray(Xb.transpose(0, 1, 3, 2, 4)).reshape(
        N_CORES, (BLOCKS_PER_CORE // 2) * 128, 2 * K * 129
    )

    lid_f = np.full(TOT, -1.0, np.float32)
    lid_f[slot] = node_loc[dst_ext].astype(np.float32)
    lid_all = (
        lid_f.reshape(N_CORES, BLOCKS_PER_CORE, K, 128)
        .transpose(0, 3, 1, 2)
        .reshape(N_CORES, 128, BLOCKS_PER_CORE * K)
        .astype(BF16)
    )
    wt = np.ascontiguousarray(W.T).astype(BF16)
    bias = np.ascontiguousarray(b.reshape(128, 1))
    ident = np.eye(128, dtype=np.float32)
    iotar = np.ascontiguousarray(
        np.broadcast_to(
            np.repeat(np.arange(128, dtype=np.float32), K), (128, 128 * K)
        )
    ).astype(BF16)

    in_maps = [
        {
            "xe": np.ascontiguousarray(xe_all[c]),
            "lid": np.ascontiguousarray(lid_all[c]),
            "wt": wt,
            "bias": bias,
            "ident": ident,
            "iotar": iotar,
        }
        for c in range(N_CORES)
    ]
    pos = node_bin * BLK + node_loc          # output column of each node
    return K, in_maps, pos


def run(edge_data, dst, W, b, trace=False, tmpdir=None):
    from concourse.bass_utils import run_bass_kernel_spmd

    K, in_maps, pos = prepare_inputs(edge_data, dst, W, b)
    nc = _get_module(K)
    res = run_bass_kernel_spmd(
        nc, in_maps, core_ids=list(range(N_CORES)), trace=trace, tmpdir=tmpdir,
    )
    full = np.concatenate(
        [res.results[c]["out"] for c in range(N_CORES)], axis=1
    )                                         # [128, 100352] bf16
    out = full.T[pos].astype(np.float32)      # un-permute -> [100000, 128]
    return np.ascontiguousarray(out), res


def kernel(edge_data, dst, W, b):
    out, _ = run(edge_data, dst, W, b, trace=False)
    return out


# revision 5
# speedup vs baseline: 2.1455x; 1.1029x over previous
"""GCN edge-aggregation kernel for 8 Trainium2 NeuronCores.

Math (see nn_GCNEdge): h = relu((segment_sum(edge_data, dst) / max(count,1)) @ W.T + b)

Strategy
--------
Host-side (sharding/layout only — all arithmetic happens on device):
  * Nodes are dealt into 784 blocks of <=128 nodes with a serpentine deal over
    degree-sorted nodes (plus a small swap-repair pass), equalizing per-block
    edge counts so every block fits K_CHUNKS*128 = 2048 edge slots.  8 cores
    x 98 blocks; the output is un-permuted on the host at the end.
  * Each edge is routed to the core/block owning its destination node; within
    a block, edges occupy sequential slots padded to K*128.
  * Edge features ship as single bf16 (the rel-err budget is 2e-2; bf16 with
    f32 PSUM accumulation lands ~1e-3) with a constant-1 column riding along
    for the degree counts.  Blocks are paired so each input DMA moves ~1 MiB.

Device-side (per core, per 128-node block):
  * One-hot of local node ids via one DVE is_equal in [p, n*K+c] layout: the
    lid operand broadcasts along n with a stride-1 innermost c axis, so all
    operands are packed 2-byte APs and the op runs in 2x DVE mode (the naive
    [p, c*128+n] layout has a stride-0 innermost broadcast -> 1x mode).
  * PE matmul-accumulate onehot.T @ [x | 1] into PSUM (chunk c of the one-hot
    is the strided slice [:, c, :]) -> per-node sums + counts,
  * mean = sums * reciprocal(count) (phantom 1e-30-count edges keep real
    zero-degree nodes finite; padding nodes produce NaNs the host drops),
  * PE transpose, then out = relu(W @ agg.T + b) in bf16 (fp32 PE matmuls run
    at 1/4 rate), output stays [out_feat, node] bf16 and is un-transposed,
    un-permuted, and cast to f32 on the host.

No collectives: output shards are disjoint.
"""

import numpy as np
import ml_dtypes

BF16 = ml_dtypes.bfloat16

N_NODES = 100000
N_EDGES = 1600000
F = 128
N_CORES = 8
BLK = 128                       # nodes per block
BLOCKS_PER_CORE = 98
TOTAL_BLOCKS = N_CORES * BLOCKS_PER_CORE        # 784
NODES_PER_CORE = BLOCKS_PER_CORE * BLK          # 12544
K_CHUNKS = 16                   # 128-edge chunks per block (capacity 2048)

_module_cache = {}


def _build_module(K):
    import concourse.mybir as mybir
    import concourse.tile as tile
    from concourse import bacc

    f32 = mybir.dt.float32
    bf16 = mybir.dt.bfloat16
    B = BLOCKS_PER_CORE
    NPAIR = B // 2
    W129 = K * 129               # per-block row width (128 feats + count col)

    nc = bacc.Bacc("TRN2", target_bir_lowering=False, debug=False)
    # xe rows are (pair, partition); each row holds both blocks' K chunks of
    # 129 bf16 values contiguously -> ~1 MiB DMAs with 8.25KB-contiguous lines.
    xe = nc.dram_tensor("xe", [NPAIR * 128, 2 * W129], bf16, kind="ExternalInput")
    lid = nc.dram_tensor("lid", [128, B * K], bf16, kind="ExternalInput")
    # iotar[p, n*K + c] = n
    iotar = nc.dram_tensor("iotar", [128, 128 * K], bf16, kind="ExternalInput")
    wt = nc.dram_tensor("wt", [128, 128], bf16, kind="ExternalInput")
    bias = nc.dram_tensor("bias", [128, 1], f32, kind="ExternalInput")
    ident = nc.dram_tensor("ident", [128, 128], f32, kind="ExternalInput")
    out = nc.dram_tensor("out", [128, B * 128], bf16, kind="ExternalOutput")

    xe_ap = xe.ap()
    out_ap = out.ap()

    with tile.TileContext(nc) as tc:
        with (
            tc.tile_pool(name="const", bufs=1) as cpool,
            tc.tile_pool(name="xp", bufs=4) as xpool,
            tc.tile_pool(name="ohp", bufs=6) as ohpool,
            tc.tile_pool(name="ep", bufs=3) as epool,
            tc.tile_pool(name="psS", bufs=4, space="PSUM") as psS,
            tc.tile_pool(name="psT", bufs=2, space="PSUM") as psT,
            tc.tile_pool(name="psO", bufs=2, space="PSUM") as psO,
        ):
            wt_t = cpool.tile([128, 128], bf16)
            nc.sync.dma_start(wt_t[:], wt.ap()[:])
            bias_t = cpool.tile([128, 1], f32)
            nc.sync.dma_start(bias_t[:], bias.ap()[:])
            id_t = cpool.tile([128, 128], f32)
            nc.sync.dma_start(id_t[:], ident.ap()[:])
            iotar_t = cpool.tile([128, 128 * K], bf16)
            nc.sync.dma_start(iotar_t[:], iotar.ap()[:])
            lid_t = cpool.tile([128, B * K], bf16)
            nc.sync.dma_start(lid_t[:], lid.ap()[:])

            group_pT = {}

            def emit_oh(b):
                oh = ohpool.tile([128, 128 * K], bf16, name=f"oh{b}", tag="oh")
                nc.vector.tensor_tensor(
                    out=oh[:].rearrange("p (n c) -> p n c", c=K),
                    in0=iotar_t[:].rearrange("p (n c) -> p n c", c=K),
                    in1=lid_t[:, b * K:(b + 1) * K].unsqueeze(1).to_broadcast(
                        [128, 128, K]
                    ),
                    op=mybir.AluOpType.is_equal,
                )
                return oh

            def emit_matmuls(b, xt, oh):
                ps = psS.tile([128, 129], f32, name=f"ps{b}", tag="ps")
                half = (b % 2) * W129
                ohc = oh[:].rearrange("p (n c) -> p c n", c=K)
                for c in range(K):
                    nc.tensor.matmul(
                        ps[:],
                        lhsT=ohc[:, c, :],
                        rhs=xt[:, half + c * 129:half + (c + 1) * 129],
                        start=(c == 0),
                        stop=(c == K - 1),
                    )
                return ps

            def emit_epilogue(b, ps):
                # Reads PSUM directly (no SBUF staging copy).  Counts live in
                # ps[:,128].  No max(count,1) guard: the host guarantees every
                # real node has count > 0 (1e-30-weight phantom edges);
                # padding nodes divide by zero -> NaN columns the host drops.
                rec = epool.tile([128, 1], f32, name=f"rec{b}", tag="rec")
                nc.vector.reciprocal(rec[:], ps[:, 128:129])
                agg = epool.tile([128, 128], f32, name=f"agg{b}", tag="agg")
                nc.scalar.activation(
                    agg[:], ps[:, 0:128],
                    mybir.ActivationFunctionType.Copy, scale=rec[:, 0:1],
                )
                j = b % 4
                if j == 0:
                    group_pT["t"] = psT.tile([128, 512], f32, name=f"pT{b}", tag="pT")
                pT = group_pT["t"]
                nc.tensor.transpose(pT[:, j * 128:(j + 1) * 128], agg[:], id_t[:])
                if j == 3 or b == B - 1:
                    g0 = (b // 4) * 4
                    gw = (b + 1 - g0) * 128
                    aggT = epool.tile([128, 512], bf16, name=f"aggT{b}", tag="aggT", bufs=2)
                    nc.scalar.copy(aggT[:, 0:gw], pT[:, 0:gw])
                    pO = psO.tile([128, 512], f32, name=f"pO{b}", tag="pO")
                    nc.tensor.matmul(
                        pO[:, 0:gw], lhsT=wt_t[:], rhs=aggT[:, 0:gw],
                        start=True, stop=True,
                    )
                    ot = epool.tile([128, 512], bf16, name=f"ot{b}", tag="ot", bufs=2)
                    nc.scalar.activation(
                        ot[:, 0:gw], pO[:, 0:gw],
                        mybir.ActivationFunctionType.Relu,
                        bias=bias_t[:, 0:1], scale=1.0,
                    )
                    # Issue output DMAs on the ACT HWDGE ring: they wait on
                    # the deep epilogue pipeline, and on the SP ring they
                    # head-of-line block the input-pair DMAs behind them.
                    nc.scalar.dma_start(out_ap[:, g0 * 128:(b + 1) * 128], ot[:, 0:gw])

            # Software-pipelined emission; per-engine queues are strict
            # in-order, so each stage's dependencies are blocks old by the
            # time its queue reaches the op:
            #   iter b:  DMA pair(b/2) | one-hot(b) | PE matmuls(b-1)
            #            | epilogue(b-3)
            pending = {}
            pending_ps = {}
            xt_of = {}
            for b in range(B):
                if b % 2 == 0:
                    q = b // 2
                    xt = xpool.tile([128, 2 * W129], bf16, name=f"xt{q}", tag="xt")
                    nc.sync.dma_start(xt[:], xe_ap[q * 128:(q + 1) * 128, :])
                    xt_of[b] = xt
                    xt_of[b + 1] = xt
                oh = emit_oh(b)
                pending[b] = (xt_of.pop(b), oh)
                if b >= 1:
                    pending_ps[b - 1] = emit_matmuls(b - 1, *pending.pop(b - 1))
                if b >= 3:
                    emit_epilogue(b - 3, pending_ps.pop(b - 3))
            last = B - 1
            pending_ps[last] = emit_matmuls(last, *pending.pop(last))
            for bb in sorted(pending_ps):
                emit_epilogue(bb, pending_ps.pop(bb))

    nc.compile()
    return nc


def _get_module(K):
    if K not in _module_cache:
        _module_cache[K] = _build_module(K)
    return _module_cache[K]


def _balance_nodes(deg):
    """Assign nodes to TOTAL_BLOCKS bins of <=128 nodes, equalizing per-bin
    edge load.  Returns (node_bin, node_loc, K)."""
    w = np.where(deg == 0, 1, deg).astype(np.int64)   # phantom edge for deg-0
    order = np.argsort(-w, kind="stable")
    S = TOTAL_BLOCKS * BLK
    idx = np.arange(S)
    r = idx // TOTAL_BLOCKS
    k = idx % TOTAL_BLOCKS
    bins = np.where(r % 2 == 0, k, TOTAL_BLOCKS - 1 - k)   # serpentine deal
    node_bin = np.empty(N_NODES, np.int64)
    node_loc = np.empty(N_NODES, np.int64)
    node_bin[order] = bins[:N_NODES]
    node_loc[order] = r[:N_NODES]
    loads = np.zeros(TOTAL_BLOCKS, np.int64)
    np.add.at(loads, node_bin, w)

    cap = K_CHUNKS * BLK
    for _ in range(2000):
        hot = int(loads.argmax())
        if loads[hot] <= cap:
            break
        cold = int(loads.argmin())
        need = int(loads[hot] - cap)
        headroom = int(cap - loads[cold])
        hot_nodes = np.nonzero(node_bin == hot)[0]
        cold_nodes = np.nonzero(node_bin == cold)[0]
        diffs = w[hot_nodes][:, None] - w[cold_nodes][None, :]
        mask = (diffs >= 1) & (diffs <= headroom)
        if not mask.any():
            break
        cand = np.where(mask, np.abs(diffs - need), 1 << 30)
        ai, bi = np.unravel_index(np.argmin(cand), cand.shape)
        na, nb = hot_nodes[ai], cold_nodes[bi]
        node_bin[na], node_bin[nb] = cold, hot
        node_loc[na], node_loc[nb] = node_loc[nb], node_loc[na]
        d = int(w[na] - w[nb])
        loads[hot] -= d
        loads[cold] += d

    K = max(K_CHUNKS, int(np.ceil(loads.max() / BLK)))
    return node_bin, node_loc, K


def prepare_inputs(edge_data, dst, W, b):
    """Host-side sharding: route each edge to the core/block owning dst."""
    edge_data = np.asarray(edge_data, dtype=np.float32)
    dst = np.asarray(dst).astype(np.int64)
    W = np.asarray(W, dtype=np.float32)
    b = np.asarray(b, dtype=np.float32)
    E = dst.shape[0]

    deg = np.bincount(dst, minlength=N_NODES)[:N_NODES]
    node_bin, node_loc, K = _balance_nodes(deg)
    RB = K * 128
    TOT = TOTAL_BLOCKS * RB

    zeros = np.nonzero(deg == 0)[0]
    dst_ext = np.concatenate([dst, zeros]) if len(zeros) else dst
    n_real = E
    Eext = dst_ext.shape[0]

    eblk = node_bin[dst_ext]
    cnt = np.bincount(eblk, minlength=TOTAL_BLOCKS)
    starts = np.zeros(TOTAL_BLOCKS, np.int64)
    np.cumsum(cnt[:-1], out=starts[1:])
    eorder = np.argsort(eblk, kind="stable")
    rank = np.empty(Eext, np.int64)
    rank[eorder] = np.arange(Eext, dtype=np.int64) - np.repeat(starts, cnt)
    slot = eblk * RB + rank

    X = np.zeros((TOT, 129), BF16)
    X[slot[:n_real], 0:128] = edge_data.astype(BF16)
    X[slot[:n_real], 128] = BF16(1.0)
    if len(zeros):
        X[slot[n_real:], 128] = BF16(1e-30)
    # [block, chunk, part, 129] -> [pair, part, 2 * chunk * 129]: each SBUF
    # partition's data is one 8.25KB contiguous HBM run, ~1MiB per DMA.
    Xb = X.reshape(TOTAL_BLOCKS, K, 128, 129).transpose(0, 2, 1, 3)
    Xb = Xb.reshape(N_CORES, BLOCKS_PER_CORE // 2, 2, 128, K * 129)
    xe_all = np.ascontiguousarray(Xb.transpose(0, 1, 3, 2, 4)).reshape(
        N_CORES, (BLOCKS_PER_CORE // 2) * 128, 2 * K * 129
    )

    lid_f = np.full(TOT, -1.0, np.float32)
    lid_f[slot] = node_loc[dst_ext].astype(np.float32)
    lid_all = (
        lid_f.reshape(N_CORES, BLOCKS_PER_CORE, K, 128)
        .transpose(0, 3, 1, 2)
        .reshape(N_CORES, 128, BLOCKS_PER_CORE * K)
        .astype(BF16)
    )
    wt = np.ascontiguousarray(W.T).astype(BF16)
    bias = np.ascontiguousarray(b.reshape(128, 1))
    ident = np.eye(128, dtype=np.float32)
    iotar = np.ascontiguousarray(
        np.broadcast_to(
            np.repeat(np.arange(128, dtype=np.float32), K), (128, 128 * K)
        )
    ).astype(BF16)

    in_maps = [
        {
            "xe": np.ascontiguousarray(xe_all[c]),
            "lid": np.ascontiguousarray(lid_all[c]),
            "wt": wt,
            "bias": bias,
            "ident": ident,
            "iotar": iotar,
        }
        for c in range(N_CORES)
    ]
    pos = node_bin * BLK + node_loc          # output column of each node
    return K, in_maps, pos


def run(edge_data, dst, W, b, trace=False, tmpdir=None):
    from concourse.bass_utils import run_bass_kernel_spmd

    K, in_maps, pos = prepare_inputs(edge_data, dst, W, b)
    nc = _get_module(K)
    res = run_bass_kernel_spmd(
        nc, in_maps, core_ids=list(range(N_CORES)), trace=trace, tmpdir=tmpdir,
    )
    full = np.concatenate(
        [res.results[c]["out"] for c in range(N_CORES)], axis=1
    )                                         # [128, 100352] bf16
    out = full.T[pos].astype(np.float32)      # un-permute -> [100000, 128]
    return np.ascontiguousarray(out), res


def kernel(edge_data, dst, W, b):
    out, _ = run(edge_data, dst, W, b, trace=False)
    return out
